# revision 1
# baseline (speedup 1.0000x reference)
"""DGL-style multi-head graph attention on 8 Trainium2 NeuronCores.

Strategy (edge/node hybrid parallelism, no collectives needed):
  * Host sorts edges by destination node and splits nodes into 8 equal
    ranges (1250 nodes/core); each core owns all edges landing in its
    range, so per-core outputs are disjoint rows of the result.
  * On each core: project q/v tables for all nodes and k for the local
    range (PE matmuls), then per 128-node tile gather q[src]/k[dst]/v[src]
    rows with dma_gather, compute scores + softmax-over-heads on DVE/ACT,
    and do the segment-sum as a one-hot matmul accumulating in PSUM.
  * Final Wo projection on PE; host concatenates the 8 row-slices.

Performance map (cost-model timeline, per core; NTFF unavailable here):
  total ~403us = phase1 ~71us (table build, 16-tile-batched writes; near
  its DMA floor) + phase2 ~333us (DMA 205 / DVE 179 / PE 166 busy).
  Phase-2 floor is ~197us of bandwidth-exact gather traffic (70MB/core).
  Known-pinned costs: the attn broadcast multiply runs at 1x DVE mode
  (stride-0 operand disqualifies 2x in every legal walk order) and each
  dma_gather pays ~2.4us SWDGE gen + DGE delay on the dependency loop
  (prepare_only/trigger_dma would hide it but needs manual semaphores
  that do not compose safely with the Tile scheduler's ordering).
  Tested-flat or negative: pool depths beyond current, B_RUN in
  {9,12,36}, PSUM rebalances, k-dedup via one-hot matmul, qT-before-v
  gather hoisting (+11us: starves the message multiply).
  HW hazard ledger: single_packet gathers beyond ~256 idxs corrupt;
  pin bufs=4 in phase 1 produced NaN on HW while passing CoreSim
  (bisected: batched writes alone are clean -- grp=8 and grp=16
  each passed 2 HW trials; the depth change was the fault).
"""

import math
from contextlib import ExitStack

import ml_dtypes
import numpy as np

import concourse.bass as bass
import concourse.mybir as mybir
import concourse.tile as tile
from concourse import bacc, bass_utils

F32 = mybir.dt.float32
BF16 = mybir.dt.bfloat16
I16 = mybir.dt.int16

N_NODES = 10000
DIM = 256
H = 8
HD = 32
NCORES = 8
NPC = N_NODES // NCORES          # nodes per core (1250)
W = 128                          # node-tile width
NT = (NPC + W - 1) // W          # node tiles per core (10)
N_CPAD = NT * W                  # padded local nodes (1280)
N_PAD = 10240                    # padded q/v table rows (80 tiles of 128)
B_RUN = 18                       # edge blocks (of 128 edges) per inner run

MULT = mybir.AluOpType.mult
ADD = mybir.AluOpType.add
ISEQ = mybir.AluOpType.is_equal
AXX = mybir.AxisListType.X

last_results = None  # BassKernelResults of the most recent run (for test.py)


def _preprocess(src, dst):
    """Sort edges by dst, bucket into (core, node-tile), pad each tile's
    edge list to a multiple of 128*B_RUN blocks shared by all cores."""
    src = np.asarray(src).astype(np.int64)
    dst = np.asarray(dst).astype(np.int64)
    order = np.argsort(dst, kind="stable")
    s_src = src[order]
    s_dst = dst[order]

    core_of = s_dst // NPC
    loc = s_dst - core_of * NPC
    tile_of = loc // W

    counts = np.zeros((NCORES, NT), np.int64)
    np.add.at(counts, (core_of, tile_of), 1)
    maxcnt = int(counts.max())
    nruns = max(1, (maxcnt + B_RUN * 128 - 1) // (B_RUN * 128))
    B = B_RUN * nruns
    EPT = B * 128  # padded edges per node tile

    src_pad = np.zeros((NCORES, NT, EPT), np.int64)
    kdst_pad = np.zeros((NCORES, NT, EPT), np.int64)     # local dst (k-table row)
    dstloc_pad = np.full((NCORES, NT, EPT), -1.0, np.float32)  # within-tile dst

    # boundaries of each (core, tile) segment in the sorted edge list
    bounds = np.array([c * NPC + t * W for c in range(NCORES) for t in range(NT)]
                      + [N_NODES], np.int64)
    seg = np.searchsorted(s_dst, bounds)
    for c in range(NCORES):
        for t in range(NT):
            i = c * NT + t
            lo, hi = seg[i], seg[i + 1]
            n = hi - lo
            assert n <= EPT
            src_pad[c, t, :n] = s_src[lo:hi]
            kdst_pad[c, t, :n] = s_dst[lo:hi] - c * NPC
            dstloc_pad[c, t, :n] = (s_dst[lo:hi] - c * NPC - t * W).astype(np.float32)

    def tile_idx(a):
        # sequence -> dma_gather layout [128, S/16]: row p holds seq[s*16 + p%16]
        seq = a.reshape(-1, 16).T.astype(np.int16)       # [16, S/16]
        return np.ascontiguousarray(np.tile(seq, (8, 1)))  # [128, S/16]

    idx_src = np.stack([tile_idx(src_pad[c]) for c in range(NCORES)])
    idx_dst = np.stack([tile_idx(kdst_pad[c]) for c in range(NCORES)])
    # [128, NT*B] with [e, t*B+b] = dstloc[t, b*128+e]
    dstloc = np.stack([
        np.ascontiguousarray(
            dstloc_pad[c].reshape(NT, B, 128).transpose(2, 0, 1).reshape(128, NT * B))
        for c in range(NCORES)])
    return B, idx_src, idx_dst, dstloc


_prog_cache = {}


def _build(B):
    import os
    skip = set(os.environ.get("KERNEL_SKIP", "").split(","))
    nruns = B // B_RUN
    SEQ = NT * B * 128
    nc = bacc.Bacc("TRN2", target_bir_lowering=False, debug=False)

    xT_d = nc.dram_tensor("xT", [DIM, N_PAD], BF16, kind="ExternalInput").ap()
    xlocT_d = nc.dram_tensor("xlocT", [DIM, N_CPAD], BF16, kind="ExternalInput").ap()
    wqvT_d = nc.dram_tensor("wqvT", [DIM, 2 * DIM], BF16, kind="ExternalInput").ap()
    wkT_d = nc.dram_tensor("wkT", [DIM, DIM], BF16, kind="ExternalInput").ap()
    woT_d = nc.dram_tensor("woT", [DIM, DIM], F32, kind="ExternalInput").ap()
    idxs_d = nc.dram_tensor("idx_src", [128, SEQ // 16], I16, kind="ExternalInput").ap()
    idxd_d = nc.dram_tensor("idx_dst", [128, SEQ // 16], I16, kind="ExternalInput").ap()
    dstloc_d = nc.dram_tensor("dstloc", [128, NT * B], BF16, kind="ExternalInput").ap()
    ident_d = nc.dram_tensor("ident", [128, 128], F32, kind="ExternalInput").ap()
    bdlo_d = nc.dram_tensor("bdlo", [128, 16], BF16, kind="ExternalInput").ap()
    bdhi_d = nc.dram_tensor("bdhi", [128, 16], BF16, kind="ExternalInput").ap()
    iota_d = nc.dram_tensor("iota", [128, 128 * B_RUN], BF16, kind="ExternalInput").ap()
    out_d = nc.dram_tensor("out", [N_CPAD, DIM], F32, kind="ExternalOutput").ap()

    with ExitStack() as ctx:
        tc = ctx.enter_context(tile.TileContext(nc))
        consts = ctx.enter_context(tc.tile_pool(name="consts", bufs=1))

        def load_w(name, d_ap):
            sb = consts.tile([128, 2, d_ap.shape[1]], d_ap.dtype, name=name)
            nc.sync.dma_start(sb[:], d_ap.rearrange("(a p) i -> p a i", p=128))
            return sb

        wqv_sb = load_w("wqv_sb", wqvT_d)
        wk_sb = load_w("wk_sb", wkT_d)
        wo_sb = load_w("wo_sb", woT_d)
        ident = consts.tile([128, 128], F32)
        nc.sync.dma_start(ident[:], ident_d)
        bdlo = consts.tile([128, 16], BF16)
        nc.sync.dma_start(bdlo[:], bdlo_d)
        bdhi = consts.tile([128, 16], BF16)
        nc.sync.dma_start(bdhi[:], bdhi_d)
        iotab_sb = consts.tile([128, 128 * B_RUN], BF16)
        nc.sync.dma_start(iotab_sb[:], iota_d)
        idxs_sb = consts.tile([128, SEQ // 16], I16)
        nc.sync.dma_start(idxs_sb[:], idxs_d)
        idxd_sb = consts.tile([128, SEQ // 16], I16)
        nc.sync.dma_start(idxd_sb[:], idxd_d)
        dstloc_sb = consts.tile([128, NT * B], BF16)
        nc.sync.dma_start(dstloc_sb[:], dstloc_d)

        dram = ctx.enter_context(tc.tile_pool(name="dram", bufs=1, space="DRAM"))
        qv_table = dram.tile([N_PAD, 2 * DIM], BF16)
        k_table = dram.tile([N_CPAD, DIM], BF16)

        # ---- phase 1: projection tables ----
        with tc.tile_pool(name="pin", bufs=3) as pin, \
             tc.tile_pool(name="pps", bufs=4, space="PSUM") as pps, \
             tc.tile_pool(name="pout", bufs=4) as pout:

            def project(src_ap, n_tiles, jobs, table, width, grp):
                assert n_tiles % grp == 0
                x4 = src_ap.rearrange("(a p) (g t w) -> p a g t w",
                                      p=128, w=128, t=grp)
                tb = table[:].rearrange("(g t p) w -> p g t w", p=128, t=grp)
                for g in range(n_tiles // grp):
                    xt = pin.tile([128, 2, grp, 128], BF16, tag="xt")
                    nc.sync.dma_start(xt[:], x4[:, :, g, :, :])
                    ob = pout.tile([128, grp, width], BF16, tag="ob")
                    for t in range(grp):
                        ps = pps.tile([128, width], F32, tag="ps")
                        nc.tensor.matmul(ps[:], xt[:, 0, t, :], jobs[:, 0, :],
                                         start=True, stop=False)
                        nc.tensor.matmul(ps[:], xt[:, 1, t, :], jobs[:, 1, :],
                                         start=False, stop=True)
                        if t % 2 == 0:
                            nc.scalar.copy(ob[:, t, :], ps[:])
                        else:
                            nc.vector.tensor_copy(ob[:, t, :], ps[:])
                    nc.scalar.dma_start(tb[:, g, :, :], ob[:])

            if "phase1" not in skip:
                project(xlocT_d, NT, wk_sb[:], k_table, DIM, 5)
                project(xT_d, N_PAD // 128, wqv_sb[:], qv_table, 2 * DIM, 16)

        # ---- phase 2: per node-tile edge processing ----
        nidx_reg = nc.alloc_register(mybir.EngineType.Pool, "nidx_reg")
        nc.gpsimd.reg_mov(nidx_reg, B_RUN * 128)
        with tc.tile_pool(name="gat", bufs=4) as gat, \
             tc.tile_pool(name="gatv", bufs=3) as gatv, \
             tc.tile_pool(name="work", bufs=2) as work, \
             tc.tile_pool(name="small", bufs=4) as small, \
             tc.tile_pool(name="hps", bufs=2, space="PSUM") as hps, \
             tc.tile_pool(name="tps", bufs=2, space="PSUM") as tps, \
             tc.tile_pool(name="stage", bufs=3) as stage:

            ne = B_RUN * 128
            ncols = B_RUN * 8
            nidx = B_RUN * 128
            for t in range(NT):
                h_ps = hps.tile([128, DIM], F32, tag="h")
                # k-gathers only need the small k_table (built first) --
                # issue the whole tile's worth up front so they can run
                # during the qv-table build and fill DMA idle slots
                kgs = []
                for r in range(nruns):
                    col0 = (t * B + r * B_RUN) * 8
                    kT_g = gat.tile([128, 2, ne], BF16, tag="kTg")
                    if "gather" not in skip:
                        nc.gpsimd.dma_gather(kT_g[:], k_table[:],
                                             idxd_sb[:, col0:col0 + ncols],
                                             nidx, nidx_reg, DIM,
                                             transpose=True, single_packet=False)
                    kgs.append(kT_g)
                ms, Ss = [], []
                for r in range(nruns):
                    col0 = (t * B + r * B_RUN) * 8
                    kT_g = kgs[r]

                    # S depends only on constants: build early so DVE has
                    # work while the gathers land
                    S = work.tile([128, 128, B_RUN], BF16, tag="S")
                    nc.vector.tensor_tensor(
                        S[:],
                        iotab_sb[:].rearrange("p (n b) -> p n b", b=B_RUN),
                        dstloc_sb[:, t * B + r * B_RUN:t * B + (r + 1) * B_RUN]
                            .unsqueeze(1).broadcast_to((128, 128, B_RUN)),
                        op=ISEQ)
                    Ss.append(S)

                    qT_g = gat.tile([128, 2, ne], BF16, tag="qTg")
                    v_g = gatv.tile([128, B_RUN, DIM], BF16, tag="vg")
                    if "gather" not in skip:
                        nc.gpsimd.dma_gather(qT_g[:], qv_table[:, 0:DIM],
                                             idxs_sb[:, col0:col0 + ncols],
                                             nidx, nidx_reg, DIM,
                                             elem_step=2 * DIM, transpose=True,
                                             single_packet=False)
                        nc.gpsimd.dma_gather(v_g[:], qv_table[:, DIM:2 * DIM],
                                             idxs_sb[:, col0:col0 + ncols],
                                             nidx, nidx_reg, DIM,
                                             elem_step=2 * DIM,
                                             single_packet=False)

                    if "compute" in skip:
                        continue
                    qkT = work.tile([128, 2, ne], BF16, tag="qkT")
                    nc.vector.tensor_tensor(qkT[:], qT_g[:], kT_g[:], op=MULT)
                    # scoresT[j, e] = sum_d qkT[d, e] * BD[d, j]  (PE, per 512-col chunk)
                    escT = work.tile([16, ne], BF16, tag="escT")
                    for e0 in range(0, ne, 512):
                        sz = min(512, ne - e0)
                        ps16 = tps.tile([16, 512], F32, tag="ps16")
                        nc.tensor.matmul(ps16[:, :sz], bdlo[:], qkT[:, 0, e0:e0 + sz],
                                         start=True, stop=False)
                        nc.tensor.matmul(ps16[:, :sz], bdhi[:], qkT[:, 1, e0:e0 + sz],
                                         start=False, stop=True)
                        nc.scalar.activation(escT[:, e0:e0 + sz], ps16[:, :sz],
                                             func=mybir.ActivationFunctionType.Exp,
                                             scale=1.0 / math.sqrt(HD))
                    esc_e = small.tile([128, B_RUN, 16], BF16, tag="esce")
                    nc.sync.dma_start(esc_e[:], escT[:], transpose=True)
                    z = small.tile([128, B_RUN], F32, tag="z")
                    nc.vector.tensor_reduce(z[:], esc_e[:, :, 0:H], axis=AXX, op=ADD)
                    zr = small.tile([128, B_RUN], F32, tag="zr")
                    nc.vector.reciprocal(zr[:], z[:])
                    attn = small.tile([128, B_RUN, H], BF16, tag="at")
                    nc.vector.tensor_tensor(
                        attn[:], esc_e[:, :, 0:H],
                        zr[:].unsqueeze(2).broadcast_to((128, B_RUN, H)), op=MULT)
                    m = work.tile([128, B_RUN, DIM], BF16, tag="m")
                    nc.vector.tensor_tensor(
                        m[:].rearrange("p b (h d) -> p b h d", h=H),
                        v_g[:].rearrange("p b (h d) -> p b h d", h=H),
                        attn[:].unsqueeze(3).broadcast_to((128, B_RUN, H, HD)),
                        op=MULT)
                    ms.append(m)
                # segment-sum after all runs' score pipelines are emitted:
                # keeps the in-order PE from blocking run r+1's score matmuls
                # behind run r's m-multiply
                if "compute" not in skip:
                    for r in range(nruns):
                        for b in range(B_RUN):
                            nc.tensor.matmul(h_ps[:], Ss[r][:, :, b],
                                             ms[r][:, b, :],
                                             start=(r == 0 and b == 0),
                                             stop=(r == nruns - 1 and b == B_RUN - 1))

                if "compute" in skip:
                    o_sb = stage.tile([128, DIM], F32, tag="o_sb")
                    nc.vector.memset(o_sb[:], 0.0)
                    nc.scalar.dma_start(out_d[t * 128:(t + 1) * 128, :], o_sb[:])
                    continue
                h_sb = stage.tile([128, DIM], F32, tag="h_sb")
                nc.scalar.copy(h_sb[:], h_ps[:])
                hT_ps = tps.tile([128, 2, 128], F32, tag="hT")
                for a in range(2):
                    nc.tensor.transpose(hT_ps[:, a, :],
                                        h_sb[:, a * 128:(a + 1) * 128], ident[:])
                hT_sb = stage.tile([128, 2, 128], F32, tag="hT_sb")
                nc.scalar.copy(hT_sb[:], hT_ps[:])
                o_ps = tps.tile([128, DIM], F32, tag="o")
                for a in range(2):
                    nc.tensor.matmul(o_ps[:], hT_sb[:, a, :], wo_sb[:, a, :],
                                     start=(a == 0), stop=(a == 1))
                o_sb = stage.tile([128, DIM], F32, tag="o_sb")
                nc.scalar.copy(o_sb[:], o_ps[:])
                nc.scalar.dma_start(out_d[t * 128:(t + 1) * 128, :], o_sb[:])

    nc.compile()
    return nc


def _bd_mat(base):
    bd = np.zeros((128, 16), np.float32)
    for d in range(128):
        bd[d, base + d // HD] = 1.0
    return bd.astype(ml_dtypes.bfloat16)


def _make_in_maps(x, Wq, Wk, Wv, Wo, idx_src, idx_dst, dstloc):
    x = np.asarray(x, np.float32)
    xp = np.zeros((N_PAD, DIM), np.float32)
    xp[:N_NODES] = x
    xT = np.ascontiguousarray(xp.T.astype(ml_dtypes.bfloat16))
    wqvT = np.ascontiguousarray(np.concatenate(
        [np.asarray(Wq, np.float32).T, np.asarray(Wv, np.float32).T],
        axis=1).astype(ml_dtypes.bfloat16))
    wkT = np.ascontiguousarray(np.asarray(Wk, np.float32).T
                               .astype(ml_dtypes.bfloat16))
    woT = np.ascontiguousarray(np.asarray(Wo, np.float32).T)
    in_maps = []
    for c in range(NCORES):
        xl = np.zeros((N_CPAD, DIM), np.float32)
        xl[:NPC] = x[c * NPC:(c + 1) * NPC]
        in_maps.append({
            "xT": xT,
            "xlocT": np.ascontiguousarray(xl.T.astype(ml_dtypes.bfloat16)),
            "wqvT": wqvT, "wkT": wkT, "woT": woT,
            "idx_src": idx_src[c], "idx_dst": idx_dst[c],
            "dstloc": dstloc[c].astype(ml_dtypes.bfloat16),
            "ident": np.eye(128, dtype=np.float32),
            "bdlo": _bd_mat(0), "bdhi": _bd_mat(4),
            "iota": np.tile(np.repeat(np.arange(128), B_RUN)
                            .astype(ml_dtypes.bfloat16), (128, 1)),
        })
    return in_maps


def kernel(x, src, dst, Wq, bq, Wk, bk, Wv, bv, Wo, bo, **_unused):
    global last_results
    assert abs(np.asarray(bq)).max() == 0 and abs(np.asarray(bk)).max() == 0 \
        and abs(np.asarray(bv)).max() == 0, "nonzero qkv biases unsupported"

    B, idx_src, idx_dst, dstloc = _preprocess(src, dst)
    if B not in _prog_cache:
        _prog_cache[B] = _build(B)
    nc = _prog_cache[B]
    in_maps = _make_in_maps(x, Wq, Wk, Wv, Wo, idx_src, idx_dst, dstloc)

    import os
    trace = bool(int(os.environ.get("KERNEL_TRACE", "0")))
    res = bass_utils.run_bass_kernel_spmd(
        nc, in_maps, core_ids=list(range(NCORES)), trace=trace)
    last_results = res

    out = np.empty((N_NODES, DIM), np.float32)
    for c in range(NCORES):
        out[c * NPC:(c + 1) * NPC] = res.results[c]["out"][:NPC]
    out += np.asarray(bo, np.float32)[None, :]
    return out



# revision 3
# speedup vs baseline: 1.5941x; 1.5941x over previous
"""DGL-style multi-head graph attention on 8 Trainium2 NeuronCores.

Iteration A over the 403us baseline:
  * Degree-balanced node->tile permutation (LPT bin packing) so every
    128-node tile has ~4000 in-edges: padded block count drops from
    B=36 to B=32 per tile (-11% on all per-edge work).
  * Inverted score matmuls: per 128-edge block, PE contracts the 256
    q*k dims (lhsT=qk block, rhs=one-hot head map [128,8]) producing
    scores edge-partitioned [128e, 8h] directly in PSUM. Replaces the
    [16, ne] score layout + DmaTranspose + 5-chunk exp with a single
    small exp per run; PE cost for scores drops ~10x.
"""

import math
from contextlib import ExitStack

import ml_dtypes
import numpy as np

import concourse.bass as bass
import concourse.mybir as mybir
import concourse.tile as tile
from concourse import bacc, bass_utils

F32 = mybir.dt.float32
BF16 = mybir.dt.bfloat16
I16 = mybir.dt.int16

N_NODES = 10000
DIM = 256
H = 8
HD = 32
NCORES = 8
W = 128                          # node-tile width
NT = 10                          # node tiles per core
NBINS = NCORES * NT              # 80 tiles total
N_CPAD = NT * W                  # padded local nodes (1280)
N_PAD = 10240                    # padded q/v table rows (80 tiles of 128)
B_RUN = 16                       # edge blocks (of 128 edges) per inner run

MULT = mybir.AluOpType.mult
ADD = mybir.AluOpType.add
ISEQ = mybir.AluOpType.is_equal
AXX = mybir.AxisListType.X

last_results = None  # BassKernelResults of the most recent run (for test.py)


def _preprocess(src, dst):
    """Degree-balanced LPT assignment of nodes to 80 tiles of 128 slots,
    then bucket edges by their dst tile and pad to B blocks of 128."""
    import heapq

    src = np.asarray(src).astype(np.int64)
    dst = np.asarray(dst).astype(np.int64)
    deg = np.bincount(dst, minlength=N_NODES)
    order = np.argsort(-deg, kind="stable")

    bin_of = np.empty(N_NODES, np.int64)
    slot_of = np.empty(N_NODES, np.int64)
    counts = np.zeros(NBINS, np.int64)
    heap = [(0, b) for b in range(NBINS)]
    heapq.heapify(heap)
    for n in order:
        while True:
            s, b = heapq.heappop(heap)
            if counts[b] < W:
                break
        bin_of[n] = b
        slot_of[n] = counts[b]
        counts[b] += 1
        if counts[b] < W:
            heapq.heappush(heap, (s + int(deg[n]), b))
    # node_of[bin, slot] = original node id (-1 for empty pad slots)
    node_of = np.full((NBINS, W), -1, np.int64)
    node_of[bin_of, slot_of] = np.arange(N_NODES)

    ebin = bin_of[dst]
    eslot = slot_of[dst]
    ecnt = np.bincount(ebin, minlength=NBINS)
    maxcnt = int(ecnt.max())
    nruns = max(1, (maxcnt + B_RUN * 128 - 1) // (B_RUN * 128))
    B = B_RUN * nruns
    EPT = B * 128  # padded edges per node tile

    order_e = np.argsort(ebin, kind="stable")
    s_src = src[order_e]
    s_slot = eslot[order_e]
    s_bin = ebin[order_e]

    src_pad = np.zeros((NCORES, NT, EPT), np.int64)
    kdst_pad = np.zeros((NCORES, NT, EPT), np.int64)     # local k-table row
    dstloc_pad = np.full((NCORES, NT, EPT), -1.0, np.float32)  # slot in tile

    bounds = np.searchsorted(s_bin, np.arange(NBINS + 1))
    for bb in range(NBINS):
        c, t = divmod(bb, NT)
        lo, hi = bounds[bb], bounds[bb + 1]
        n = hi - lo
        assert n <= EPT
        src_pad[c, t, :n] = s_src[lo:hi]
        kdst_pad[c, t, :n] = s_slot[lo:hi]  # rank-local row (tile t's rank)
        dstloc_pad[c, t, :n] = s_slot[lo:hi].astype(np.float32)

    def tile_idx(a):
        # sequence -> dma_gather layout [128, S/16]: row p holds seq[s*16 + p%16]
        seq = a.reshape(-1, 16).T.astype(np.int16)       # [16, S/16]
        return np.ascontiguousarray(np.tile(seq, (8, 1)))  # [128, S/16]

    idx_src = np.stack([tile_idx(src_pad[c]) for c in range(NCORES)])
    idx_dst = np.stack([tile_idx(kdst_pad[c]) for c in range(NCORES)])
    # [128, NT*B] with [e, t*B+b] = dstloc[t, b*128+e]
    dstloc = np.stack([
        np.ascontiguousarray(
            dstloc_pad[c].reshape(NT, B, 128).transpose(2, 0, 1).reshape(128, NT * B))
        for c in range(NCORES)])
    return B, idx_src, idx_dst, dstloc, node_of


_prog_cache = {}


def _build(B):
    import os
    skip = set(os.environ.get("KERNEL_SKIP", "").split(","))
    RB = [int(x) for x in os.environ.get(
        "KERNEL_RINGS", "6,4,4,3,2,4,2").split(",")]  # gatk,gatq,gatv,S,qk,m,stage
    VAR = set(os.environ.get("KERNEL_VAR", "").split(","))
    nruns = B // B_RUN
    SEQ = NT * B * 128
    nc = bacc.Bacc("TRN2", target_bir_lowering=False, debug=False)

    xT_d = nc.dram_tensor("xT", [DIM, N_PAD], BF16, kind="ExternalInput").ap()
    xlocT_d = nc.dram_tensor("xlocT", [DIM, N_CPAD], BF16, kind="ExternalInput").ap()
    wqvT_d = nc.dram_tensor("wqvT", [DIM, 2 * DIM], BF16, kind="ExternalInput").ap()
    wkT_d = nc.dram_tensor("wkT", [DIM, DIM], BF16, kind="ExternalInput").ap()
    woT_d = nc.dram_tensor("woT", [DIM, DIM], BF16, kind="ExternalInput").ap()
    idxs_d = nc.dram_tensor("idx_src", [128, SEQ // 16], I16, kind="ExternalInput").ap()
    idxd_d = nc.dram_tensor("idx_dst", [128, SEQ // 16], I16, kind="ExternalInput").ap()
    dstloc_d = nc.dram_tensor("dstloc", [128, NT * B], BF16, kind="ExternalInput").ap()
    bd8lo_d = nc.dram_tensor("bd8lo", [128, 8], BF16, kind="ExternalInput").ap()
    bd8hi_d = nc.dram_tensor("bd8hi", [128, 8], BF16, kind="ExternalInput").ap()
    out_d = nc.dram_tensor("out", [N_CPAD, DIM], BF16, kind="ExternalOutput").ap()

    with ExitStack() as ctx:
        tc = ctx.enter_context(tile.TileContext(nc))
        consts = ctx.enter_context(tc.tile_pool(name="consts", bufs=1))

        def load_w(name, d_ap):
            sb = consts.tile([128, 2, d_ap.shape[1]], d_ap.dtype, name=name)
            nc.sync.dma_start(sb[:], d_ap.rearrange("(a p) i -> p a i", p=128))
            return sb

        wk_sb = load_w("wk_sb", wkT_d)
        idxd_sb = consts.tile([128, SEQ // 16], I16)
        nc.sync.dma_start(idxd_sb[:], idxd_d)
        wqv_sb = load_w("wqv_sb", wqvT_d)
        idxs_sb = consts.tile([128, SEQ // 16], I16)
        nc.sync.dma_start(idxs_sb[:], idxs_d)
        wo_sb = load_w("wo_sb", woT_d)
        bd8lo = consts.tile([128, 8], BF16)
        bd8hi = consts.tile([128, 8], BF16)
        iotab_sb = consts.tile([128, 128 * B_RUN], BF16)
        dstloc_sb = consts.tile([128, NT * B], BF16)

        # SBUF-resident k table (row i -> partition i%128, rank i//128)
        k_table = consts.tile([128, NT, DIM], BF16)

        dram = ctx.enter_context(tc.tile_pool(name="dram", bufs=1, space="DRAM"))
        q_table = dram.tile([N_PAD, DIM], BF16)
        v_table = dram.tile([N_PAD, DIM], BF16)

        nidx_reg = nc.alloc_register(mybir.EngineType.Pool, "nidx_reg")
        nc.gpsimd.reg_mov(nidx_reg, B_RUN * 128)
        gatk = ctx.enter_context(tc.tile_pool(name="gatk", bufs=RB[0]))

        # ---- phase 1: projection tables ----
        with tc.tile_pool(name="pin", bufs=3) as pin, \
             tc.tile_pool(name="pps", bufs=4, space="PSUM") as pps, \
             tc.tile_pool(name="pout", bufs=3) as pout, \
             tc.tile_pool(name="pov", bufs=5) as pov:

            if "phase1" not in skip:
                # k: local projection straight into the SBUF table
                xk = xlocT_d.rearrange("(a p) (g t w) -> p a g t w",
                                       p=128, w=128, t=5)
                for g in range(2):
                    xt = pin.tile([128, 2, 5, 128], BF16, tag="xt")
                    nc.sync.dma_start(xt[:], xk[:, :, g, :, :])
                    for t in range(5):
                        ps = pps.tile([128, DIM], F32, tag="psq")
                        nc.tensor.matmul(ps[:], xt[:, 0, t, :], wk_sb[:, 0, :],
                                         start=True, stop=False)
                        nc.tensor.matmul(ps[:], xt[:, 1, t, :], wk_sb[:, 1, :],
                                         start=False, stop=True)
                        if t % 2 == 0:
                            nc.scalar.copy(k_table[:, g * 5 + t, :], ps[:])
                        else:
                            nc.vector.tensor_copy(k_table[:, g * 5 + t, :], ps[:])

                # separate q and v DRAM tables; q written first per group so
                # q gathers can start before the v table completes
                GRP = 16
                x4 = xT_d.rearrange("(a p) (g t w) -> p a g t w",
                                    p=128, w=128, t=GRP)
                tbq = q_table[:].rearrange("(g t p) w -> p g t w", p=128, t=GRP)
                tbv = v_table[:].rearrange("(g t p) w -> p g t w", p=128, t=GRP)
                obvs = []
                for g in range(N_PAD // 128 // GRP):
                    xt = pin.tile([128, 2, GRP, 128], BF16, tag="xt")
                    nc.sync.dma_start(xt[:], x4[:, :, g, :, :])
                    obq = pout.tile([128, GRP, DIM], BF16, tag="obq")
                    obv = pov.tile([128, GRP, DIM], BF16, tag="obv")
                    for t in range(GRP):
                        psq = pps.tile([128, DIM], F32, tag="psq")
                        nc.tensor.matmul(psq[:], xt[:, 0, t, :],
                                         wqv_sb[:, 0, 0:DIM],
                                         start=True, stop=False)
                        nc.tensor.matmul(psq[:], xt[:, 1, t, :],
                                         wqv_sb[:, 1, 0:DIM],
                                         start=False, stop=True)
                        psv = pps.tile([128, DIM], F32, tag="psv")
                        nc.tensor.matmul(psv[:], xt[:, 0, t, :],
                                         wqv_sb[:, 0, DIM:2 * DIM],
                                         start=True, stop=False)
                        nc.tensor.matmul(psv[:], xt[:, 1, t, :],
                                         wqv_sb[:, 1, DIM:2 * DIM],
                                         start=False, stop=True)
                        nc.scalar.copy(obq[:, t, :], psq[:])
                        nc.vector.tensor_copy(obv[:, t, :], psv[:])
                    nc.scalar.dma_start(tbq[:, g, :, :], obq[:])
                    obvs.append((g, obv))
                # v writes deferred below all q writes so the q table (and
                # with it the whole phase-2 gather pipeline) is ready first
                for g, obv in obvs:
                    nc.scalar.dma_start(tbv[:, g, :, :], obv[:])

            nc.sync.dma_start(bd8lo[:], bd8lo_d)
            nc.sync.dma_start(bd8hi[:], bd8hi_d)
            nc.gpsimd.iota(iotab_sb[:].rearrange("p (n b) -> p n b", b=B_RUN),
                           [[1, 128], [0, B_RUN]], channel_multiplier=0,
                           allow_small_or_imprecise_dtypes=True)
            nc.sync.dma_start(dstloc_sb[:], dstloc_d)

        # ---- phase 2: per node-tile edge processing ----
        with tc.tile_pool(name="gatq", bufs=RB[1]) as gatq, \
             tc.tile_pool(name="gatv", bufs=RB[2]) as gatv, \
             tc.tile_pool(name="spool", bufs=RB[3]) as spool, \
             tc.tile_pool(name="qkpool", bufs=RB[4]) as qkpool, \
             tc.tile_pool(name="mpool", bufs=RB[5]) as mpool, \
             tc.tile_pool(name="small", bufs=4) as small, \
             tc.tile_pool(name="hps", bufs=2, space="PSUM") as hps, \
             tc.tile_pool(name="scps", bufs=2, space="PSUM") as scps, \
             tc.tile_pool(name="tps", bufs=2, space="PSUM") as tps, \
             tc.tile_pool(name="stage", bufs=RB[6]) as stage:

            ne = B_RUN * 128
            ncols = B_RUN * 8
            nidx = B_RUN * 128
            for t in range(NT):
                h0_ps = hps.tile([128, 128], F32, tag="h0")
                h1_ps = hps.tile([128, 128], F32, tag="h1")
                h_ps = [h0_ps, h1_ps]
                # k-gathers only need the small k_table (built first) --
                # issue the whole tile's worth up front so they can run
                # during the qv-table build and fill DMA idle slots
                kgs = []
                for r in range(nruns):
                    col0 = (t * B + r * B_RUN) * 8
                    kT_g = gatk.tile([128, 2, ne], BF16, tag="kTg")
                    if "gather" not in skip:
                        nc.gpsimd.dma_gather(kT_g[:], k_table[:, t, :],
                                             idxd_sb[:, col0:col0 + ncols],
                                             nidx, nidx_reg, DIM,
                                             transpose=True, single_packet=False,
                                             sbuf_tokens_per_rank=128,
                                             sbuf_free_dim_per_rank=2 * DIM)
                    kgs.append(kT_g)
                # stage 1: S builds (consts only) + q/v gather issuance
                Ss, qgs, vgs = [], [], []
                for r in range(nruns):
                    col0 = (t * B + r * B_RUN) * 8
                    S = spool.tile([128, 128, B_RUN], BF16, tag="S")
                    nc.vector.tensor_tensor(
                        S[:],
                        iotab_sb[:].rearrange("p (n b) -> p n b", b=B_RUN),
                        dstloc_sb[:, t * B + r * B_RUN:t * B + (r + 1) * B_RUN]
                            .unsqueeze(1).broadcast_to((128, 128, B_RUN)),
                        op=ISEQ)
                    Ss.append(S)
                    qT_g = gatq.tile([128, 2, ne], BF16, tag="qTg")
                    v_g = gatv.tile([128, B_RUN, DIM], BF16, tag="vg")
                    if "gather" not in skip:
                        nc.gpsimd.dma_gather(qT_g[:], q_table[:],
                                             idxs_sb[:, col0:col0 + ncols],
                                             nidx, nidx_reg, DIM,
                                             transpose=True,
                                             single_packet=False)
                        nc.gpsimd.dma_gather(v_g[:], v_table[:],
                                             idxs_sb[:, col0:col0 + ncols],
                                             nidx, nidx_reg, DIM,
                                             single_packet=False)
                    qgs.append(qT_g)
                    vgs.append(v_g)

                # stage 2: qk products + scores + exp for all runs, so the
                # in-order DVE frees gather rings before the softmax chain
                escs = []
                for r in range(nruns):
                    if "compute" in skip:
                        continue
                    qkT = qkpool.tile([128, 2, ne], BF16, tag="qkT")
                    nc.vector.tensor_tensor(qkT[:], qgs[r][:], kgs[r][:], op=MULT)
                    # scores edge-partitioned: sc[e, b, h] = sum_d qkT[d, e]*BD8[d, h]
                    sc_ps = scps.tile([128, B_RUN, H], F32, tag="sc")
                    for b in range(B_RUN):
                        nc.tensor.matmul(sc_ps[:, b, :],
                                         qkT[:, 0, b * 128:(b + 1) * 128],
                                         bd8lo[:], start=True, stop=False)
                        nc.tensor.matmul(sc_ps[:, b, :],
                                         qkT[:, 1, b * 128:(b + 1) * 128],
                                         bd8hi[:], start=False, stop=True)
                    esc_e = small.tile([128, B_RUN, H], BF16, tag="esce")
                    nc.scalar.activation(esc_e[:], sc_ps[:],
                                         func=mybir.ActivationFunctionType.Exp,
                                         scale=1.0 / math.sqrt(HD))
                    escs.append(esc_e)

                # stage 3: softmax normalize + messages
                ms = []
                for r in range(nruns):
                    if "compute" in skip:
                        continue
                    esc_e = escs[r]
                    z = small.tile([128, B_RUN], F32, tag="z")
                    nc.vector.tensor_reduce(z[:], esc_e[:], axis=AXX, op=ADD)
                    zr = small.tile([128, B_RUN], F32, tag="zr")
                    nc.vector.reciprocal(zr[:], z[:])
                    # attn materialized in PAIRS so the m-multiply's
                    # broadcast operand has a stride-1 innermost dim (2x DVE)
                    attn = small.tile([128, B_RUN, H, 2], BF16, tag="at")
                    nc.vector.tensor_tensor(
                        attn[:],
                        esc_e[:].unsqueeze(3).broadcast_to((128, B_RUN, H, 2)),
                        zr[:].unsqueeze(2).unsqueeze(3)
                            .broadcast_to((128, B_RUN, H, 2)), op=MULT)
                    m = mpool.tile([128, B_RUN, DIM], BF16, tag="m")
                    chunks = ((2, 2, 4, 8) if "mgrad" in VAR else (4, 4, 4, 4))
                    c0 = 0
                    for MC in chunks:  # chunked so seg matmuls become ready
                        nc.vector.tensor_tensor(  # early; keeps PE streak hot
                            m[:, c0:c0 + MC, :]
                                .rearrange("p b (h x two) -> p b h x two",
                                           h=H, two=2),
                            vgs[r][:, c0:c0 + MC, :]
                                .rearrange("p b (h x two) -> p b h x two",
                                           h=H, two=2),
                            attn[:, c0:c0 + MC, :, :].unsqueeze(3)
                                .broadcast_to((128, MC, H, HD // 2, 2)),
                            op=MULT)
                        c0 += MC
                    ms.append(m)
                # segment-sum after all runs' score pipelines are emitted:
                # keeps the in-order PE from blocking run r+1's score matmuls
                # behind run r's m-multiply
                if "compute" not in skip:
                    # swapped operands: hT[d, j] = sum_e m[e, d] S[e, j] --
                    # the h tile comes out pre-transposed for the Wo matmul
                    for r in range(nruns):
                        for b in range(B_RUN):
                            for a in range(2):
                                nc.tensor.matmul(
                                    h_ps[a][:],
                                    ms[r][:, b, a * 128:(a + 1) * 128],
                                    Ss[r][:, :, b],
                                    start=(r == 0 and b == 0),
                                    stop=(r == nruns - 1 and b == B_RUN - 1))

                if "compute" in skip:
                    o_sb = stage.tile([128, DIM], BF16, tag="o_sb")
                    nc.vector.memset(o_sb[:], 0.0)
                    nc.scalar.dma_start(out_d[t * 128:(t + 1) * 128, :], o_sb[:])
                    continue
                hT_sb = stage.tile([128, 2, 128], BF16, tag="hT_sb")
                nc.scalar.copy(hT_sb[:, 0, :], h_ps[0][:])
                nc.scalar.copy(hT_sb[:, 1, :], h_ps[1][:])
                o_ps = tps.tile([128, DIM], F32, tag="o")
                for a in range(2):
                    nc.tensor.matmul(o_ps[:], hT_sb[:, a, :], wo_sb[:, a, :],
                                     start=(a == 0), stop=(a == 1))
                o_sb = stage.tile([128, DIM], BF16, tag="o_sb")
                nc.scalar.copy(o_sb[:], o_ps[:])
                wout = nc.scalar.dma_start(out_d[t * 128:(t + 1) * 128, :], o_sb[:])
                wout.bass_priority = 1 << 24  # drain after all gathers

    nc.compile()
    return nc


def _bd8_mat(base):
    bd = np.zeros((128, 8), np.float32)
    for d in range(128):
        bd[d, base + d // HD] = 1.0
    return bd.astype(ml_dtypes.bfloat16)


def _make_in_maps(x, Wq, Wk, Wv, Wo, idx_src, idx_dst, dstloc, node_of):
    x = np.asarray(x, np.float32)
    xp = np.zeros((N_PAD, DIM), np.float32)
    xp[:N_NODES] = x
    xT = np.ascontiguousarray(xp.T.astype(ml_dtypes.bfloat16))
    wqvT = np.ascontiguousarray(np.concatenate(
        [np.asarray(Wq, np.float32).T, np.asarray(Wv, np.float32).T],
        axis=1).astype(ml_dtypes.bfloat16))
    wkT = np.ascontiguousarray(np.asarray(Wk, np.float32).T
                               .astype(ml_dtypes.bfloat16))
    woT = np.ascontiguousarray(np.asarray(Wo, np.float32).T
                               .astype(ml_dtypes.bfloat16))
    in_maps = []
    for c in range(NCORES):
        # xloc row (t*128 + s) = x[node_of[c*NT + t, s]] (zeros for pads)
        nodes = node_of[c * NT:(c + 1) * NT].reshape(-1)
        xl = np.zeros((N_CPAD, DIM), np.float32)
        valid = nodes >= 0
        xl[valid] = x[nodes[valid]]
        in_maps.append({
            "xT": xT,
            "xlocT": np.ascontiguousarray(xl.T.astype(ml_dtypes.bfloat16)),
            "wqvT": wqvT, "wkT": wkT, "woT": woT,
            "idx_src": idx_src[c], "idx_dst": idx_dst[c],
            "dstloc": dstloc[c].astype(ml_dtypes.bfloat16),
            "bd8lo": _bd8_mat(0), "bd8hi": _bd8_mat(4),
        })
    return in_maps


def kernel(x, src, dst, Wq, bq, Wk, bk, Wv, bv, Wo, bo, **_unused):
    global last_results
    assert abs(np.asarray(bq)).max() == 0 and abs(np.asarray(bk)).max() == 0 \
        and abs(np.asarray(bv)).max() == 0, "nonzero qkv biases unsupported"

    B, idx_src, idx_dst, dstloc, node_of = _preprocess(src, dst)
    if B not in _prog_cache:
        _prog_cache[B] = _build(B)
    nc = _prog_cache[B]
    in_maps = _make_in_maps(x, Wq, Wk, Wv, Wo, idx_src, idx_dst, dstloc, node_of)

    import os
    trace = bool(int(os.environ.get("KERNEL_TRACE", "0")))
    res = bass_utils.run_bass_kernel_spmd(
        nc, in_maps, core_ids=list(range(NCORES)), trace=trace)
    last_results = res

    out = np.empty((N_NODES, DIM), np.float32)
    for c in range(NCORES):
        nodes = node_of[c * NT:(c + 1) * NT].reshape(-1)
        valid = nodes >= 0
        out[nodes[valid]] = res.results[c]["out"][valid]
    out += np.asarray(bo, np.float32)[None, :]
    return out


# revision 8
# speedup vs baseline: 1.7007x; 1.0669x over previous
"""DGL-style multi-head graph attention on 8 Trainium2 NeuronCores.

Iteration A over the 403us baseline:
  * Degree-balanced node->tile permutation (LPT bin packing) so every
    128-node tile has ~4000 in-edges: padded block count drops from
    B=36 to B=32 per tile (-11% on all per-edge work).
  * Inverted score matmuls: per 128-edge block, PE contracts the 256
    q*k dims (lhsT=qk block, rhs=one-hot head map [128,8]) producing
    scores edge-partitioned [128e, 8h] directly in PSUM. Replaces the
    [16, ne] score layout + DmaTranspose + 5-chunk exp with a single
    small exp per run; PE cost for scores drops ~10x.
"""

import math
from contextlib import ExitStack

import ml_dtypes
import numpy as np

import concourse.bass as bass
import concourse.mybir as mybir
import concourse.tile as tile
from concourse import bacc, bass_utils

F32 = mybir.dt.float32
BF16 = mybir.dt.bfloat16
I16 = mybir.dt.int16

N_NODES = 10000
DIM = 256
H = 8
HD = 32
NCORES = 8
W = 128                          # node-tile width
NT = 10                          # node tiles per core
NBINS = NCORES * NT              # 80 tiles total
N_CPAD = NT * W                  # padded local nodes (1280)
N_PAD = 10240                    # padded q/v table rows (80 tiles of 128)
B_RUN = 16                       # edge blocks (of 128 edges) per inner run

MULT = mybir.AluOpType.mult
ADD = mybir.AluOpType.add
ISEQ = mybir.AluOpType.is_equal
AXX = mybir.AxisListType.X

last_results = None  # BassKernelResults of the most recent run (for test.py)


def _preprocess(src, dst):
    """Degree-aware assignment of nodes to 80 tiles of <=128 slots with
    skewed per-tile-index edge targets: tile 0..8 of each core hold ~4080
    in-edges (32 blocks), tile 9 holds ~3280 (26 blocks). Identical block
    structure across cores keeps the program SPMD."""
    import heapq

    src = np.asarray(src).astype(np.int64)
    dst = np.asarray(dst).astype(np.int64)
    deg = np.bincount(dst, minlength=N_NODES)
    order = np.argsort(-deg, kind="stable")

    def assign(targets, caps):
        bin_of = np.empty(N_NODES, np.int64)
        slot_of = np.empty(N_NODES, np.int64)
        counts = np.zeros(NBINS, np.int64)
        esum = np.zeros(NBINS, np.int64)
        # heap keyed by (esum - target): most-underfull bin first
        heap = [(-targets[b % NT], b) for b in range(NBINS)]
        heapq.heapify(heap)
        for n in order:
            d = int(deg[n])
            spill = []
            while True:
                gap, b = heapq.heappop(heap)
                if counts[b] < W and esum[b] + d <= caps[b % NT]:
                    break
                if counts[b] < W:
                    spill.append((gap, b))  # edge-cap full; may take 0-deg
            for it in spill:
                heapq.heappush(heap, it)
            bin_of[n] = b
            slot_of[n] = counts[b]
            counts[b] += 1
            esum[b] += d
            if counts[b] < W:
                heapq.heappush(heap, (esum[b] - targets[b % NT], b))
        return bin_of, slot_of, esum

    TB = [32] * (NT - 1) + [26]
    targets = [4080] * (NT - 1) + [3280]
    caps = [tb * 128 for tb in TB]
    try:
        bin_of, slot_of, esum = assign(targets, caps)
    except IndexError:
        # infeasible for this degree distribution: uniform fallback
        TB = [32] * NT
        targets = [4000] * NT
        caps = [tb * 128 for tb in TB]
        bin_of, slot_of, esum = assign(targets, caps)
    for b in range(NBINS):
        assert esum[b] <= caps[b % NT], (b, esum[b])

    node_of = np.full((NBINS, W), -1, np.int64)
    node_of[bin_of, slot_of] = np.arange(N_NODES)

    ebin = bin_of[dst]
    eslot = slot_of[dst]
    order_e = np.argsort(ebin, kind="stable")
    s_src = src[order_e]
    s_slot = eslot[order_e]
    s_bin = ebin[order_e]

    off = np.concatenate([[0], np.cumsum(TB)]) * 128  # edge offset per tile
    TOT = int(off[-1])                                # padded edges per core
    src_pad = np.zeros((NCORES, TOT), np.int64)
    kdst_pad = np.zeros((NCORES, TOT), np.int64)
    dstloc_pad = np.full((NCORES, TOT), -1.0, np.float32)

    bounds = np.searchsorted(s_bin, np.arange(NBINS + 1))
    for bb in range(NBINS):
        c, t = divmod(bb, NT)
        lo, hi = bounds[bb], bounds[bb + 1]
        n = hi - lo
        assert n <= off[t + 1] - off[t]
        o = int(off[t])
        src_pad[c, o:o + n] = s_src[lo:hi]
        kdst_pad[c, o:o + n] = s_slot[lo:hi]  # rank-local row (tile t's rank)
        dstloc_pad[c, o:o + n] = s_slot[lo:hi].astype(np.float32)

    def tile_idx(a):
        # sequence -> dma_gather layout [128, S/16]: row p holds seq[s*16 + p%16]
        seq = a.reshape(-1, 16).T.astype(np.int16)       # [16, S/16]
        return np.ascontiguousarray(np.tile(seq, (8, 1)))  # [128, S/16]

    idx_src = np.stack([tile_idx(src_pad[c]) for c in range(NCORES)])
    idx_dst = np.stack([tile_idx(kdst_pad[c]) for c in range(NCORES)])
    # dstloc in block-major gather layout: [e, blk] = dstloc[blk*128 + e]
    nblk = TOT // 128
    dstloc = np.stack([
        np.ascontiguousarray(
            dstloc_pad[c].reshape(nblk, 128).T)
        for c in range(NCORES)])
    return tuple(TB), idx_src, idx_dst, dstloc, node_of


_prog_cache = {}


def _build(TB):
    import os
    skip = set(os.environ.get("KERNEL_SKIP", "").split(","))
    RB = [int(x) for x in os.environ.get(
        "KERNEL_RINGS", "6,4,4,3,2,4,2").split(",")]  # gatk,gatq,gatv,S,qk,m,stage
    VAR = set(os.environ.get("KERNEL_VAR", "").split(","))
    NBLK = sum(TB)
    OFFB = [sum(TB[:t]) for t in range(NT)]
    GRAD = os.environ.get("KERNEL_GRAD", "8,4,2")
    def tile_segs(t):
        segs, b0 = [], 0
        while b0 < TB[t]:
            nb = min(B_RUN, TB[t] - b0)
            segs.append((b0, nb))
            b0 += nb
        if t == NT - 1 and GRAD:
            tail = [int(x) for x in GRAD.split(",")]
            if sum(tail) < TB[t]:
                segs = [(0, TB[t] - sum(tail))]
                b0 = TB[t] - sum(tail)
                for nb in tail:
                    segs.append((b0, nb))
                    b0 += nb
        return segs
    SEG_SIZES = sorted({nb for t in range(NT) for _, nb in tile_segs(t)})
    SEQ = NBLK * 128
    nc = bacc.Bacc("TRN2", target_bir_lowering=False, debug=False)

    xT_d = nc.dram_tensor("xT", [DIM, N_PAD], BF16, kind="ExternalInput").ap()
    xlocT_d = nc.dram_tensor("xlocT", [DIM, N_CPAD], BF16, kind="ExternalInput").ap()
    wqvT_d = nc.dram_tensor("wqvT", [DIM, 2 * DIM], BF16, kind="ExternalInput").ap()
    wkT_d = nc.dram_tensor("wkT", [DIM, DIM], BF16, kind="ExternalInput").ap()
    woT_d = nc.dram_tensor("woT", [DIM, DIM], BF16, kind="ExternalInput").ap()
    idxs_d = nc.dram_tensor("idx_src", [128, SEQ // 16], I16, kind="ExternalInput").ap()
    idxd_d = nc.dram_tensor("idx_dst", [128, SEQ // 16], I16, kind="ExternalInput").ap()
    dstloc_d = nc.dram_tensor("dstloc", [128, NBLK], BF16, kind="ExternalInput").ap()
    bd8lo_d = nc.dram_tensor("bd8lo", [128, 8], BF16, kind="ExternalInput").ap()
    bd8hi_d = nc.dram_tensor("bd8hi", [128, 8], BF16, kind="ExternalInput").ap()
    out_d = nc.dram_tensor("out", [N_CPAD, DIM], BF16, kind="ExternalOutput").ap()

    with ExitStack() as ctx:
        tc = ctx.enter_context(tile.TileContext(nc))
        consts = ctx.enter_context(tc.tile_pool(name="consts", bufs=1))

        def load_w(name, d_ap):
            sb = consts.tile([128, 2, d_ap.shape[1]], d_ap.dtype, name=name)
            nc.sync.dma_start(sb[:], d_ap.rearrange("(a p) i -> p a i", p=128))
            return sb

        wk_sb = load_w("wk_sb", wkT_d)
        idxd_sb = consts.tile([128, SEQ // 16], I16)
        nc.sync.dma_start(idxd_sb[:], idxd_d)
        wqv_sb = load_w("wqv_sb", wqvT_d)
        idxs_sb = consts.tile([128, SEQ // 16], I16)
        nc.sync.dma_start(idxs_sb[:], idxs_d)
        wo_sb = load_w("wo_sb", woT_d)
        bd8lo = consts.tile([128, 8], BF16)
        bd8hi = consts.tile([128, 8], BF16)
        iotab_sb = consts.tile([128, 128 * B_RUN], BF16)
        dstloc_sb = consts.tile([128, NBLK], BF16)

        # SBUF-resident k table (row i -> partition i%128, rank i//128)
        k_table = consts.tile([128, NT, DIM], BF16)

        dram = ctx.enter_context(tc.tile_pool(name="dram", bufs=1, space="DRAM"))
        q_table = dram.tile([N_PAD, DIM], BF16)
        v_table = dram.tile([N_PAD, DIM], BF16)

        nidx_regs = {}
        for nb in SEG_SIZES:
            nidx_regs[nb] = nc.alloc_register(mybir.EngineType.Pool,
                                              f"nidx_reg{nb}")
            nc.gpsimd.reg_mov(nidx_regs[nb], nb * 128)
        gatk = ctx.enter_context(tc.tile_pool(name="gatk", bufs=RB[0]))

        # ---- phase 1: projection tables ----
        with tc.tile_pool(name="pin", bufs=3) as pin, \
             tc.tile_pool(name="pps", bufs=4, space="PSUM") as pps, \
             tc.tile_pool(name="pout", bufs=3) as pout, \
             tc.tile_pool(name="pov", bufs=5) as pov:

            if "phase1" not in skip:
                # k: local projection straight into the SBUF table
                xk = xlocT_d.rearrange("(a p) (g t w) -> p a g t w",
                                       p=128, w=128, t=5)
                for g in range(2):
                    xt = pin.tile([128, 2, 5, 128], BF16, tag="xt")
                    nc.sync.dma_start(xt[:], xk[:, :, g, :, :])
                    for t in range(5):
                        ps = pps.tile([128, DIM], F32, tag="psq")
                        nc.tensor.matmul(ps[:], xt[:, 0, t, :], wk_sb[:, 0, :],
                                         start=True, stop=False)
                        nc.tensor.matmul(ps[:], xt[:, 1, t, :], wk_sb[:, 1, :],
                                         start=False, stop=True)
                        if t % 2 == 0:
                            nc.scalar.copy(k_table[:, g * 5 + t, :], ps[:])
                        else:
                            nc.vector.tensor_copy(k_table[:, g * 5 + t, :], ps[:])

                # separate q and v DRAM tables; q written first per group so
                # q gathers can start before the v table completes
                GRP = 16
                x4 = xT_d.rearrange("(a p) (g t w) -> p a g t w",
                                    p=128, w=128, t=GRP)
                tbq = q_table[:].rearrange("(g t p) w -> p g t w", p=128, t=GRP)
                tbv = v_table[:].rearrange("(g t p) w -> p g t w", p=128, t=GRP)
                obvs = []
                for g in range(N_PAD // 128 // GRP):
                    xt = pin.tile([128, 2, GRP, 128], BF16, tag="xt")
                    nc.sync.dma_start(xt[:], x4[:, :, g, :, :])
                    obq = pout.tile([128, GRP, DIM], BF16, tag="obq")
                    obv = pov.tile([128, GRP, DIM], BF16, tag="obv")
                    for t in range(GRP):
                        psq = pps.tile([128, DIM], F32, tag="psq")
                        nc.tensor.matmul(psq[:], xt[:, 0, t, :],
                                         wqv_sb[:, 0, 0:DIM],
                                         start=True, stop=False)
                        nc.tensor.matmul(psq[:], xt[:, 1, t, :],
                                         wqv_sb[:, 1, 0:DIM],
                                         start=False, stop=True)
                        psv = pps.tile([128, DIM], F32, tag="psv")
                        nc.tensor.matmul(psv[:], xt[:, 0, t, :],
                                         wqv_sb[:, 0, DIM:2 * DIM],
                                         start=True, stop=False)
                        nc.tensor.matmul(psv[:], xt[:, 1, t, :],
                                         wqv_sb[:, 1, DIM:2 * DIM],
                                         start=False, stop=True)
                        if t % 2 == 0:
                            nc.scalar.copy(obq[:, t, :], psq[:])
                            nc.vector.tensor_copy(obv[:, t, :], psv[:])
                        else:
                            nc.vector.tensor_copy(obq[:, t, :], psq[:])
                            nc.scalar.copy(obv[:, t, :], psv[:])
                    nc.scalar.dma_start(tbq[:, g, :, :], obq[:])
                    obvs.append((g, obv))
                # v writes deferred below all q writes so the q table (and
                # with it the whole phase-2 gather pipeline) is ready first
                for g, obv in obvs:
                    nc.sync.dma_start(tbv[:, g, :, :], obv[:])

            nc.sync.dma_start(bd8lo[:], bd8lo_d)
            nc.sync.dma_start(bd8hi[:], bd8hi_d)
            nc.gpsimd.iota(iotab_sb[:].rearrange("p (n b) -> p n b", b=B_RUN),
                           [[1, 128], [0, B_RUN]], channel_multiplier=0,
                           allow_small_or_imprecise_dtypes=True)
            nc.sync.dma_start(dstloc_sb[:], dstloc_d)

        # ---- phase 2: per node-tile edge processing ----
        with tc.tile_pool(name="gatq", bufs=RB[1]) as gatq, \
             tc.tile_pool(name="gatv", bufs=RB[2]) as gatv, \
             tc.tile_pool(name="spool", bufs=RB[3]) as spool, \
             tc.tile_pool(name="qkpool", bufs=RB[4]) as qkpool, \
             tc.tile_pool(name="mpool", bufs=RB[5]) as mpool, \
             tc.tile_pool(name="small", bufs=4) as small, \
             tc.tile_pool(name="hps", bufs=2, space="PSUM") as hps, \
             tc.tile_pool(name="scps", bufs=2, space="PSUM") as scps, \
             tc.tile_pool(name="tps", bufs=2, space="PSUM") as tps, \
             tc.tile_pool(name="stage", bufs=RB[6]) as stage:

            for t in range(NT):
                segs = tile_segs(t)
                # [128, 2, 512]: each a-half owns a full PSUM bank so the
                # two interleaved accumulation groups don't collide
                hT_ps = hps.tile([128, 2, 512], F32, tag="h")
                h_ps = [hT_ps[:, 0, 0:128], hT_ps[:, 1, 0:128]]
                kgs = []
                for blk0, nb in segs:
                    col0 = (OFFB[t] + blk0) * 8
                    kT_g = gatk.tile([128, 2, nb * 128], BF16, tag="kTg")
                    if "gather" not in skip:
                        nc.gpsimd.dma_gather(kT_g[:], k_table[:, t, :],
                                             idxd_sb[:, col0:col0 + nb * 8],
                                             nb * 128, nidx_regs[nb], DIM,
                                             transpose=True, single_packet=False,
                                             sbuf_tokens_per_rank=128,
                                             sbuf_free_dim_per_rank=2 * DIM)
                    kgs.append(kT_g)
                # stage 1: S builds (consts only) + q/v gather issuance
                Ss, qgs, vgs = [], [], []
                for blk0, nb in segs:
                    col0 = (OFFB[t] + blk0) * 8
                    S = spool.tile([128, 128, nb], BF16, tag="S")
                    nc.vector.tensor_tensor(
                        S[:],
                        iotab_sb[:].rearrange("p (n b) -> p n b", b=B_RUN)
                            [:, :, 0:nb],
                        dstloc_sb[:, OFFB[t] + blk0:OFFB[t] + blk0 + nb]
                            .unsqueeze(1).broadcast_to((128, 128, nb)),
                        op=ISEQ)
                    Ss.append(S)
                    qT_g = gatq.tile([128, 2, nb * 128], BF16, tag="qTg")
                    v_g = gatv.tile([128, nb, DIM], BF16, tag="vg")
                    if "gather" not in skip:
                        nc.gpsimd.dma_gather(qT_g[:], q_table[:],
                                             idxs_sb[:, col0:col0 + nb * 8],
                                             nb * 128, nidx_regs[nb], DIM,
                                             transpose=True,
                                             single_packet=False)
                        nc.gpsimd.dma_gather(v_g[:], v_table[:],
                                             idxs_sb[:, col0:col0 + nb * 8],
                                             nb * 128, nidx_regs[nb], DIM,
                                             single_packet=False)
                    qgs.append(qT_g)
                    vgs.append(v_g)

                if "compute" in skip:
                    o_sb = stage.tile([128, DIM], BF16, tag="o_sb")
                    nc.vector.memset(o_sb[:], 0.0)
                    nc.scalar.dma_start(out_d[t * 128:(t + 1) * 128, :], o_sb[:])
                    continue

                # stage 2: qk products + scores + exp for all runs, so the
                # in-order DVE frees gather rings before the softmax chain
                escs = []
                for r, (blk0, nb) in enumerate(segs):
                    qkT = qkpool.tile([128, 2, nb * 128], BF16, tag="qkT")
                    nc.vector.tensor_tensor(qkT[:], qgs[r][:], kgs[r][:], op=MULT)
                    sc_ps = scps.tile([128, nb, H], F32, tag="sc")
                    for b in range(nb):
                        nc.tensor.matmul(sc_ps[:, b, :],
                                         qkT[:, 0, b * 128:(b + 1) * 128],
                                         bd8lo[:], start=True, stop=False)
                        nc.tensor.matmul(sc_ps[:, b, :],
                                         qkT[:, 1, b * 128:(b + 1) * 128],
                                         bd8hi[:], start=False, stop=True)
                    esc_e = small.tile([128, nb, H], BF16, tag="esce")
                    nc.scalar.activation(esc_e[:], sc_ps[:],
                                         func=mybir.ActivationFunctionType.Exp,
                                         scale=1.0 / math.sqrt(HD))
                    escs.append(esc_e)

                # stage 3: softmax normalize + messages
                ms = []
                for r, (blk0, nb) in enumerate(segs):
                    esc_e = escs[r]
                    z = small.tile([128, nb], F32, tag="z")
                    nc.vector.tensor_reduce(z[:], esc_e[:], axis=AXX, op=ADD)
                    zr = small.tile([128, nb], F32, tag="zr")
                    nc.vector.reciprocal(zr[:], z[:])
                    # attn materialized in PAIRS so the m-multiply's
                    # broadcast operand has a stride-1 innermost dim (2x DVE)
                    attn = small.tile([128, nb, H, 2], BF16, tag="at")
                    nc.vector.tensor_tensor(
                        attn[:],
                        esc_e[:].unsqueeze(3).broadcast_to((128, nb, H, 2)),
                        zr[:].unsqueeze(2).unsqueeze(3)
                            .broadcast_to((128, nb, H, 2)), op=MULT)
                    m = mpool.tile([128, nb, DIM], BF16, tag="m")
                    for c0 in range(0, nb, 4):  # chunked so seg matmuls become
                        MC = min(4, nb - c0)    # ready early (PE streak stays hot)
                        nc.vector.tensor_tensor(
                            m[:, c0:c0 + MC, :]
                                .rearrange("p b (h x two) -> p b h x two",
                                           h=H, two=2),
                            vgs[r][:, c0:c0 + MC, :]
                                .rearrange("p b (h x two) -> p b h x two",
                                           h=H, two=2),
                            attn[:, c0:c0 + MC, :, :].unsqueeze(3)
                                .broadcast_to((128, MC, H, HD // 2, 2)),
                            op=MULT)
                    ms.append(m)
                if "compute" not in skip:
                    # swapped operands: hT[d, j] = sum_e m[e, d] S[e, j] --
                    # the h tile comes out pre-transposed for the Wo matmul
                    nseg = len(segs)
                    for r, (blk0, nb) in enumerate(segs):
                        for b in range(nb):
                            for a in range(2):
                                nc.tensor.matmul(
                                    h_ps[a],
                                    ms[r][:, b, a * 128:(a + 1) * 128],
                                    Ss[r][:, :, b],
                                    start=(r == 0 and b == 0),
                                    stop=(r == nseg - 1 and b == nb - 1))

                hT_sb = stage.tile([128, 2, 128], BF16, tag="hT_sb")
                nc.scalar.copy(hT_sb[:], hT_ps[:, :, 0:128])
                o_ps = tps.tile([128, DIM], F32, tag="o")
                for a in range(2):
                    nc.tensor.matmul(o_ps[:], hT_sb[:, a, :], wo_sb[:, a, :],
                                     start=(a == 0), stop=(a == 1))
                o_sb = stage.tile([128, DIM], BF16, tag="o_sb")
                nc.scalar.copy(o_sb[:], o_ps[:])
                wout = nc.sync.dma_start(out_d[t * 128:(t + 1) * 128, :], o_sb[:])
                wout.bass_priority = 1 << 24  # drain after all gathers

    nc.compile()
    return nc


def _bd8_mat(base):
    bd = np.zeros((128, 8), np.float32)
    for d in range(128):
        bd[d, base + d // HD] = 1.0
    return bd.astype(ml_dtypes.bfloat16)


def _make_in_maps(x, Wq, Wk, Wv, Wo, idx_src, idx_dst, dstloc, node_of):
    x = np.asarray(x, np.float32)
    xp = np.zeros((N_PAD, DIM), np.float32)
    xp[:N_NODES] = x
    xT = np.ascontiguousarray(xp.T.astype(ml_dtypes.bfloat16))
    wqvT = np.ascontiguousarray(np.concatenate(
        [np.asarray(Wq, np.float32).T, np.asarray(Wv, np.float32).T],
        axis=1).astype(ml_dtypes.bfloat16))
    wkT = np.ascontiguousarray(np.asarray(Wk, np.float32).T
                               .astype(ml_dtypes.bfloat16))
    woT = np.ascontiguousarray(np.asarray(Wo, np.float32).T
                               .astype(ml_dtypes.bfloat16))
    in_maps = []
    for c in range(NCORES):
        # xloc row (t*128 + s) = x[node_of[c*NT + t, s]] (zeros for pads)
        nodes = node_of[c * NT:(c + 1) * NT].reshape(-1)
        xl = np.zeros((N_CPAD, DIM), np.float32)
        valid = nodes >= 0
        xl[valid] = x[nodes[valid]]
        in_maps.append({
            "xT": xT,
            "xlocT": np.ascontiguousarray(xl.T.astype(ml_dtypes.bfloat16)),
            "wqvT": wqvT, "wkT": wkT, "woT": woT,
            "idx_src": idx_src[c], "idx_dst": idx_dst[c],
            "dstloc": dstloc[c].astype(ml_dtypes.bfloat16),
            "bd8lo": _bd8_mat(0), "bd8hi": _bd8_mat(4),
        })
    return in_maps


def kernel(x, src, dst, Wq, bq, Wk, bk, Wv, bv, Wo, bo, **_unused):
    global last_results
    assert abs(np.asarray(bq)).max() == 0 and abs(np.asarray(bk)).max() == 0 \
        and abs(np.asarray(bv)).max() == 0, "nonzero qkv biases unsupported"

    TB, idx_src, idx_dst, dstloc, node_of = _preprocess(src, dst)
    if TB not in _prog_cache:
        _prog_cache[TB] = _build(TB)
    nc = _prog_cache[TB]
    in_maps = _make_in_maps(x, Wq, Wk, Wv, Wo, idx_src, idx_dst, dstloc, node_of)

    import os
    trace = bool(int(os.environ.get("KERNEL_TRACE", "0")))
    res = bass_utils.run_bass_kernel_spmd(
        nc, in_maps, core_ids=list(range(NCORES)), trace=trace)
    last_results = res

    out = np.empty((N_NODES, DIM), np.float32)
    for c in range(NCORES):
        nodes = node_of[c * NT:(c + 1) * NT].reshape(-1)
        valid = nodes >= 0
        out[nodes[valid]] = res.results[c]["out"][valid]
    out += np.asarray(bo, np.float32)[None, :]
    return out
